# revision 24
# baseline (speedup 1.0000x reference)
"""Trainium2 Bass kernel v2 for nn_CameraAwareSparseBlock.

Key changes vs v1 (both for tunnel-shipped bytes and device exec):
  - Ship the pair-packed features UNREPLICATED ([16, fwinp] u32, 1/8 the
    bytes); replicate to 128 partitions on-device with 3 doubling SBUF DMAs.
  - ONE shared neighbor-index tensor for both convs: conv1 and conv2 use the
    same window positions / chunk geometry / tap grouping (8 taps per gather
    inst, 16-partition groups), so the same [128, cols] i16 tensor feeds both.
    Conv2 achieves 16-row tap groups via ap_gather d=2 (each index fetches two
    u32 pair-words = 4 bf16 channels per partition).
  - Conv2 source is built once per chunk at [16, src, 4]-bf16 (BN1 affine +
    ReLU applied at 16 partitions) then partition-doubled to 128.
  - Output y stays channel-major [64, 25088] bf16 (no on-device transpose);
    host transposes/casts. Halves output + donated-zero shipping.
  - All small constants merged into two blobs (wbf / wf32) to cut per-tensor
    dispatch overhead.

Per-core shipped bytes: ~4.6 MB in + 3.2 MB zero + 3.2 MB out
(vs v1: 20.9 MB in + 6.4 MB zero + 6.4 MB out).
"""

import numpy as np
import ml_dtypes
from contextlib import ExitStack
from dataclasses import dataclass

import concourse.bass as bass
import concourse.tile as tile
from concourse import bacc, mybir
from concourse.bass_utils import run_bass_kernel_spmd

BF16 = ml_dtypes.bfloat16
F32 = np.float32
AF = mybir.ActivationFunctionType
ALU = mybir.AluOpType
AX = mybir.AxisListType

EPS = 1e-5

G_TAPS = [8, 8, 8, 3]   # tap grouping per gather inst (27 taps)


@dataclass(frozen=True)
class Geo2:
    n: int = 200000
    n_cores: int = 8
    til: int = 512
    m: int = 1024      # neighbor position margin
    ct: int = 20       # tiles per chunk
    nch: int = 3       # chunks
    bt: int = 2        # tiles per gather block
    yw: int = 25088    # output width (>= own, mult of til)
    c1: int = 32
    c2: int = 64
    k: int = 27
    cam: int = 256

    @property
    def own(self):
        return self.n // self.n_cores

    @property
    def chunk(self):
        return self.ct * self.til

    @property
    def win(self):
        return self.nch * self.chunk

    @property
    def src(self):
        return self.chunk + 2 * self.m + 1

    @property
    def fwinp(self):
        return (self.nch - 1) * self.chunk + self.src

    @property
    def o0(self):
        # window column of first own position
        return 2 * self.m

    @property
    def idx_cols(self):
        ncol = self.til // 16
        return self.nch * (self.ct // self.bt) * 4 * (self.bt * ncol)

    def check(self):
        assert self.m % self.til == 0
        assert self.ct % self.bt == 0
        assert self.win >= self.own + 4 * self.m
        assert self.o0 + self.yw <= self.win
        assert self.src * 2 <= 32768          # conv2 d=2 gather constraint
        assert self.src - 1 < 32767           # int16 idx
        assert self.yw % self.til == 0 and self.yw >= self.own
        assert self.n % self.n_cores == 0


GEO2 = Geo2()


# ---------------------------------------------------------------------------
# Device program
# ---------------------------------------------------------------------------

def build_module(g: Geo2, single: bool = False):
    g.check()
    nc = bacc.Bacc("TRN2", target_bir_lowering=False, debug=False,
                   num_devices=(1 if single else g.n_cores))
    u32, i16, bf, f32 = (mybir.dt.uint32, mybir.dt.int16,
                         mybir.dt.bfloat16, mybir.dt.float32)
    til = g.til
    ncol = til // 16
    nblk = g.ct // g.bt
    bw = g.bt * til
    blk_cols = 4 * g.bt * ncol        # idx cols per block
    ch_cols = nblk * blk_cols         # idx cols per chunk

    fpu = nc.dram_tensor("fpu", [16, g.fwinp], u32, kind="ExternalInput")
    idx = nc.dram_tensor("idx", [128, g.idx_cols], i16, kind="ExternalInput")
    # wbf cols: 0:512 wpk1 | 512:1536 wpk2 | 1536:1664 wrt (rows 0:16)
    wbf = nc.dram_tensor("wbf", [128, 1664], bf, kind="ExternalInput")
    # wf32 cols: 0:64 wcs | 64:128 wcsh | 128:132 camt | rows 0:64:
    #   132:136 bsel | 136:138 g1|be1 | 138:140 g2|be2 | 140:141 br
    wf32 = nc.dram_tensor("wf32", [g.cam + 1, 141], f32, kind="ExternalInput")
    y = nc.dram_tensor("y", [64, g.yw], bf, kind="ExternalOutput")

    hh = nc.dram_tensor("hh", [64, g.win], bf, kind="Internal")
    h2pre = nc.dram_tensor("h2pre", [64, g.win], bf, kind="Internal")
    ar1i = nc.dram_tensor("ar1i", [64, 2], f32, kind="Internal")
    ar1o = nc.dram_tensor("ar1o", [64, 2], f32, kind="Internal")
    ar2i = nc.dram_tensor("ar2i", [64, 2], f32, kind="Internal")
    ar2o = nc.dram_tensor("ar2o", [64, 2], f32, kind="Internal")
    c1d = nc.dram_tensor("c1d", [64, 2], f32, kind="Internal")

    groups = [list(range(g.n_cores))]
    inv_n = 1.0 / float(g.n)

    # own-position mask over window tiles (for BN stats)
    ts0 = g.o0 // til
    ts1 = (g.o0 + g.own) // til
    rem = (g.o0 + g.own) % til
    ncols_s = (ts1 - ts0) + (1 if rem else 0)

    with tile.TileContext(nc) as tc:
        with ExitStack() as ctx:
            cpool = ctx.enter_context(tc.tile_pool(name="const", bufs=1))
            bigp = ctx.enter_context(tc.tile_pool(name="bigsrc", bufs=1))
            ldp = ctx.enter_context(tc.tile_pool(name="ld", bufs=2))
            gatp = ctx.enter_context(tc.tile_pool(name="gat", bufs=3))
            gtp = ctx.enter_context(tc.tile_pool(name="gt", bufs=2))
            idxp = ctx.enter_context(tc.tile_pool(name="idx", bufs=2))
            stgp = ctx.enter_context(tc.tile_pool(name="stg", bufs=3))
            psp = ctx.enter_context(tc.tile_pool(name="ps", bufs=6,
                                                 space="PSUM"))

            # ---- constants ----
            w1sb = cpool.tile([128, 512], bf)
            nc.sync.dma_start(w1sb[:], wbf.ap()[:, 0:512])
            w2sb = cpool.tile([128, 1024], bf)
            nc.sync.dma_start(w2sb[:], wbf.ap()[:, 512:1536])
            wrsb = cpool.tile([16, 128], bf)
            nc.sync.dma_start(wrsb[:], wbf.ap()[0:16, 1536:1664])
            gb1sb = cpool.tile([64, 2], f32)
            nc.sync.dma_start(gb1sb[:], wf32.ap()[0:64, 136:138])
            gb2sb = cpool.tile([64, 2], f32)
            nc.sync.dma_start(gb2sb[:], wf32.ap()[0:64, 138:140])
            brsb = cpool.tile([64, 1], f32)
            nc.sync.dma_start(brsb[:], wf32.ap()[0:64, 140:141])
            bsel_sb = cpool.tile([64, 4], f32)
            nc.sync.dma_start(bsel_sb[:], wf32.ap()[0:64, 132:136])

            # ---- P0: camera MLP -> film scale/shift [64,1] ----
            fs = cpool.tile([64, 1], f32)
            fsh = cpool.tile([64, 1], f32)
            for cb, out in ((0, fs), (64, fsh)):
                pc = psp.tile([64, 4], f32, space="PSUM", tag="ps")
                done = 0
                total = g.cam + 1
                first = True
                while done < total:
                    kk = min(128, total - done)
                    wchunk = stgp.tile([128, 64], f32, tag="wchunk")
                    nc.sync.dma_start(wchunk[0:kk, :],
                                      wf32.ap()[done:done + kk, cb:cb + 64])
                    cchunk = stgp.tile([128, 4], f32, tag="cchunk")
                    nc.sync.dma_start(cchunk[0:kk, :],
                                      wf32.ap()[done:done + kk, 128:132])
                    nc.tensor.matmul(pc[:], wchunk[0:kk, :], cchunk[0:kk, :],
                                     start=first, stop=(done + kk == total))
                    first = False
                    done += kk
                csb = stgp.tile([64, 4], f32, tag="csb")
                nc.vector.tensor_copy(csb[:], pc[:])
                tmp = stgp.tile([64, 4], f32, tag="csb")
                nc.vector.tensor_tensor(out=tmp[:], in0=csb[:], in1=bsel_sb[:],
                                        op=ALU.mult)
                nc.vector.tensor_reduce(out=out[:], in_=tmp[:], axis=AX.X,
                                        op=ALU.add)

            # ---- P-A: conv1 over window ----
            s1sum = cpool.tile([64, 64], f32)
            s1sq = cpool.tile([64, 64], f32)

            def stage(wt_idx, ps, sh, ssum, ssq):
                """Copy psum->bf16 stage + BN stats accumulation on own cols."""
                if ts0 <= wt_idx < ts1:
                    col = wt_idx - ts0
                    nc.scalar.activation(sh[:], ps[:], AF.Copy,
                                         accum_out=ssum[:, col:col + 1])
                    scr = stgp.tile([64, til], bf, tag="scr")
                    nc.scalar.activation(scr[:], ps[:], AF.Square,
                                         accum_out=ssq[:, col:col + 1])
                elif wt_idx == ts1 and rem:
                    col = ts1 - ts0
                    nc.scalar.activation(sh[:, 0:rem], ps[:, 0:rem], AF.Copy,
                                         accum_out=ssum[:, col:col + 1])
                    nc.scalar.activation(sh[:, rem:til], ps[:, rem:til],
                                         AF.Copy)
                    scr = stgp.tile([64, til], bf, tag="scr")
                    nc.scalar.activation(scr[:, 0:rem], ps[:, 0:rem],
                                         AF.Square,
                                         accum_out=ssq[:, col:col + 1])
                else:
                    nc.scalar.activation(sh[:], ps[:], AF.Copy)

            for j in range(g.nch):
                fsrc = bigp.tile([128, 2 * g.src], u32, tag="bigsrc")
                nc.sync.dma_start(
                    fsrc[0:16, 0:g.src - 1],
                    fpu.ap()[:, j * g.chunk:j * g.chunk + g.src - 1])
                nc.vector.memset(fsrc[0:16, g.src - 1:g.src], 0)
                nc.sync.dma_start(fsrc[16:32, 0:g.src], fsrc[0:16, 0:g.src])
                nc.sync.dma_start(fsrc[32:64, 0:g.src], fsrc[0:32, 0:g.src])
                nc.sync.dma_start(fsrc[64:128, 0:g.src], fsrc[0:64, 0:g.src])
                idxc = idxp.tile([128, ch_cols], i16, tag="idx")
                nc.sync.dma_start(
                    idxc[:], idx.ap()[:, j * ch_cols:(j + 1) * ch_cols])

                pend3 = None
                for b in range(nblk):
                    bg = j * nblk + b
                    if bg > 27:
                        # window tiles > 55 feed no needed conv2 position
                        continue
                    gobs = []
                    for gi in range(3):
                        go = gatp.tile([128, 2 * bw], u32, tag="gg")
                        col0 = b * blk_cols + gi * g.bt * ncol
                        nc.gpsimd.ap_gather(
                            out_ap=go[0:128, 0:bw],
                            in_ap=fsrc[0:128, 0:g.src],
                            idxs_ap=idxc[0:128, col0:col0 + g.bt * ncol],
                            channels=128, num_elems=g.src, d=1, num_idxs=bw)
                        gobs.append(go)
                    if bg % 2 == 0:
                        # merged tail: rows 0:48 this block, 48:96 next block
                        gt = gtp.tile([128, 2 * bw], u32, tag="gt")
                        col0 = b * blk_cols + 3 * g.bt * ncol
                        nc.gpsimd.ap_gather(
                            out_ap=gt[0:112, 0:bw],
                            in_ap=fsrc[0:112, 0:g.src],
                            idxs_ap=idxc[0:112, col0:col0 + g.bt * ncol],
                            channels=112, num_elems=g.src, d=1, num_idxs=bw)
                        pend3, tlo = gt, 0
                    else:
                        gt, tlo = pend3, 64
                    shw = stgp.tile([64, bw], bf, tag="sh")
                    for tl in range(g.bt):
                        wt = j * g.ct + b * g.bt + tl
                        ps = psp.tile([64, til], f32, space="PSUM", tag="ps")
                        for gi in range(3):
                            gob = gobs[gi][:].bitcast(bf).rearrange(
                                "p (n two) -> p n two", two=2)
                            for par in range(2):
                                cb = (gi * 2 + par) * 64
                                nc.tensor.matmul(
                                    ps[:], w1sb[0:128, cb:cb + 64],
                                    gob[0:128, tl * til:(tl + 1) * til, par],
                                    start=(gi == 0 and par == 0), stop=False)
                        gob = gt[:].bitcast(bf).rearrange(
                            "p (n two) -> p n two", two=2)
                        for par in range(2):
                            cb = (3 * 2 + par) * 64
                            nc.tensor.matmul(
                                ps[:], w1sb[tlo:tlo + 48, cb:cb + 64],
                                gob[tlo:tlo + 48, tl * til:(tl + 1) * til,
                                    par],
                                start=False, stop=(par == 1))
                        stage(wt, ps, shw[:, tl * til:(tl + 1) * til],
                              s1sum, s1sq)
                    w0 = (j * g.ct + b * g.bt) * til
                    nc.sync.dma_start(hh.ap()[:, w0:w0 + bw], shw[:])

            # ---- AR1 + BN1 coeffs ----
            st1 = cpool.tile([64, 2], f32)
            nc.vector.tensor_reduce(out=st1[:, 0:1], in_=s1sum[:, 0:ncols_s],
                                    axis=AX.X, op=ALU.add)
            nc.vector.tensor_reduce(out=st1[:, 1:2], in_=s1sq[:, 0:ncols_s],
                                    axis=AX.X, op=ALU.add)
            nc.sync.dma_start(ar1i.ap(), st1[:])
            if single:
                nc.sync.dma_start(ar1o.ap(), st1[:])
            else:
                nc.gpsimd.collective_compute(
                    "AllReduce", ALU.add, replica_groups=groups,
                    ins=[ar1i.ap()], outs=[ar1o.ap()])
            ar1sb = cpool.tile([64, 2], f32)
            nc.sync.dma_start(ar1sb[:], ar1o.ap())

            epssb = cpool.tile([64, 1], f32)
            nc.vector.memset(epssb[:], EPS)

            def bn_coeffs(arsb, gbsb, tag):
                mean = stgp.tile([64, 1], f32, tag=tag)
                nc.scalar.mul(mean[:], arsb[:, 0:1], inv_n)
                ex2 = stgp.tile([64, 1], f32, tag=tag)
                nc.scalar.mul(ex2[:], arsb[:, 1:2], inv_n)
                var = stgp.tile([64, 1], f32, tag=tag)
                nc.vector.tensor_tensor(out=var[:], in0=mean[:], in1=mean[:],
                                        op=ALU.mult)
                nc.vector.tensor_tensor(out=var[:], in0=ex2[:], in1=var[:],
                                        op=ALU.subtract)
                sd = stgp.tile([64, 1], f32, tag=tag)
                nc.scalar.activation(sd[:], var[:], AF.Sqrt, bias=epssb[:])
                d = stgp.tile([64, 1], f32, tag=tag)
                nc.vector.reciprocal(d[:], sd[:])
                a = stgp.tile([64, 1], f32, tag=tag)
                nc.vector.tensor_tensor(out=a[:], in0=d[:], in1=gbsb[:, 0:1],
                                        op=ALU.mult)
                b = stgp.tile([64, 1], f32, tag=tag)
                nc.vector.tensor_tensor(out=b[:], in0=mean[:], in1=a[:],
                                        op=ALU.mult)
                nc.vector.tensor_tensor(out=b[:], in0=gbsb[:, 1:2], in1=b[:],
                                        op=ALU.subtract)
                return a, b

            a1, b1c = bn_coeffs(ar1sb, gb1sb, "bnc1")
            c1sb = cpool.tile([64, 2], f32)
            nc.vector.tensor_copy(c1sb[:, 0:1], a1[:])
            nc.vector.tensor_copy(c1sb[:, 1:2], b1c[:])
            nc.sync.dma_start(c1d.ap(), c1sb[:])
            # c1 coeffs rewrapped [16, 8]: row r col 2q+s = (a|b)(ch 4r+q)
            c1w = cpool.tile([16, 8], f32)
            nc.sync.dma_start(c1w[:],
                              bass.AP(tensor=c1d, offset=0,
                                      ap=[[8, 16], [2, 4], [1, 2]]))

            # ---- P-C: conv2 over window ----
            s2sum = cpool.tile([64, 64], f32)
            s2sq = cpool.tile([64, 64], f32)
            HSUB = 4096
            for j in range(g.nch):
                h2p = bigp.tile([128, 2 * g.src], u32, tag="bigsrc")
                h2pb = h2p[:].bitcast(bf).rearrange("p (n four) -> p n four",
                                                    four=4)
                base_w = j * g.chunk - g.m   # window col of src elem 0
                # clamp to the computed hh range [0, win); out-of-range src
                # elements only feed unused window-edge outputs.
                lo = max(0, -base_w)
                hi = min(g.src - 1, 28 * 2 * til - base_w)
                if lo > 0:
                    nc.vector.memset(h2p[0:16, 0:2 * lo], 0)
                if hi < g.src - 1:
                    nc.vector.memset(h2p[0:16, 2 * hi:2 * (g.src - 1)], 0)
                done = lo
                while done < hi:
                    width = min(HSUB, hi - done)
                    for q in range(4):
                        hq = ldp.tile([16, HSUB], bf, tag="hq")
                        # hh rows q,4+q,...,60+q; cols base_w+done ...
                        srcap = bass.AP(
                            tensor=hh, offset=q * g.win + base_w + done,
                            ap=[[4 * g.win, 16], [1, width]])
                        nc.sync.dma_start(hq[:, 0:width], srcap)
                        nc.scalar.activation(
                            h2pb[0:16, done:done + width, q], hq[:, 0:width],
                            AF.Relu, bias=c1w[:, 2 * q + 1:2 * q + 2],
                            scale=c1w[:, 2 * q:2 * q + 1])
                    done += width
                nc.vector.memset(h2p[0:16, 2 * (g.src - 1):2 * g.src], 0)
                nc.sync.dma_start(h2p[16:32, :], h2p[0:16, :])
                nc.sync.dma_start(h2p[32:64, :], h2p[0:32, :])
                nc.sync.dma_start(h2p[64:128, :], h2p[0:64, :])

                idxc = idxp.tile([128, ch_cols], i16, tag="idx")
                nc.sync.dma_start(
                    idxc[:], idx.ap()[:, j * ch_cols:(j + 1) * ch_cols])

                pend3 = None
                for b in range(nblk):
                    bg = j * nblk + b
                    if not (2 <= bg <= 26):
                        # only window tiles 4..53 feed P-D / BN2 stats
                        continue
                    gobs = []
                    for gi in range(3):
                        go = gatp.tile([128, 2 * bw], u32, tag="gg")
                        col0 = b * blk_cols + gi * g.bt * ncol
                        nc.gpsimd.ap_gather(
                            out_ap=go[0:128, 0:2 * bw],
                            in_ap=h2p[0:128, :],
                            idxs_ap=idxc[0:128, col0:col0 + g.bt * ncol],
                            channels=128, num_elems=g.src, d=2, num_idxs=bw)
                        gobs.append(go)
                    if bg % 2 == 0 and bg + 1 <= 26:
                        gt = gtp.tile([128, 2 * bw], u32, tag="gt")
                        col0 = b * blk_cols + 3 * g.bt * ncol
                        nc.gpsimd.ap_gather(
                            out_ap=gt[0:112, 0:2 * bw],
                            in_ap=h2p[0:112, :],
                            idxs_ap=idxc[0:112, col0:col0 + g.bt * ncol],
                            channels=112, num_elems=g.src, d=2, num_idxs=bw)
                        pend3, tlo = gt, 0
                    elif bg % 2 == 1 and pend3 is not None:
                        gt, tlo = pend3, 64
                    else:
                        # solo block (bg=26): classic 48-row tail gather
                        gt = gtp.tile([128, 2 * bw], u32, tag="gt")
                        col0 = b * blk_cols + 3 * g.bt * ncol
                        nc.gpsimd.ap_gather(
                            out_ap=gt[0:48, 0:2 * bw],
                            in_ap=h2p[0:48, :],
                            idxs_ap=idxc[0:48, col0:col0 + g.bt * ncol],
                            channels=48, num_elems=g.src, d=2, num_idxs=bw)
                        tlo = 0
                    sh2 = stgp.tile([64, bw], bf, tag="sh2")
                    for tl in range(g.bt):
                        wt = j * g.ct + b * g.bt + tl
                        ps2 = psp.tile([64, til], f32, space="PSUM", tag="ps")
                        nmm = 0
                        for gi in range(3):
                            gob = gobs[gi][:].bitcast(bf).rearrange(
                                "p (n four) -> p n four", four=4)
                            for q in range(4):
                                cb = (gi * 4 + q) * 64
                                nmm += 1
                                nc.tensor.matmul(
                                    ps2[:], w2sb[0:128, cb:cb + 64],
                                    gob[0:128, tl * til:(tl + 1) * til, q],
                                    start=(nmm == 1), stop=False)
                        gob = gt[:].bitcast(bf).rearrange(
                            "p (n four) -> p n four", four=4)
                        for q in range(4):
                            cb = (3 * 4 + q) * 64
                            nc.tensor.matmul(
                                ps2[:], w2sb[tlo:tlo + 48, cb:cb + 64],
                                gob[tlo:tlo + 48, tl * til:(tl + 1) * til, q],
                                start=False, stop=(q == 3))
                        stage(wt, ps2, sh2[:, tl * til:(tl + 1) * til],
                              s2sum, s2sq)
                    w0 = (j * g.ct + b * g.bt) * til
                    nc.sync.dma_start(h2pre.ap()[:, w0:w0 + bw], sh2[:])

            # ---- AR2 + BN2*FiLM coeffs ----
            st2 = cpool.tile([64, 2], f32)
            nc.vector.tensor_reduce(out=st2[:, 0:1], in_=s2sum[:, 0:ncols_s],
                                    axis=AX.X, op=ALU.add)
            nc.vector.tensor_reduce(out=st2[:, 1:2], in_=s2sq[:, 0:ncols_s],
                                    axis=AX.X, op=ALU.add)
            nc.sync.dma_start(ar2i.ap(), st2[:])
            if single:
                nc.sync.dma_start(ar2o.ap(), st2[:])
            else:
                nc.gpsimd.collective_compute(
                    "AllReduce", ALU.add, replica_groups=groups,
                    ins=[ar2i.ap()], outs=[ar2o.ap()])
            ar2sb = cpool.tile([64, 2], f32)
            nc.sync.dma_start(ar2sb[:], ar2o.ap())
            a2r, b2r = bn_coeffs(ar2sb, gb2sb, "bnc2")
            fs1 = cpool.tile([64, 1], f32)
            nc.vector.tensor_scalar(out=fs1[:], in0=fs[:], scalar1=1.0,
                                    scalar2=None, op0=ALU.add)
            a2 = cpool.tile([64, 1], f32)
            nc.vector.tensor_tensor(out=a2[:], in0=a2r[:], in1=fs1[:],
                                    op=ALU.mult)
            b2 = cpool.tile([64, 1], f32)
            nc.vector.tensor_tensor(out=b2[:], in0=b2r[:], in1=fs1[:],
                                    op=ALU.mult)
            nc.vector.tensor_tensor(out=b2[:], in0=b2[:], in1=fsh[:],
                                    op=ALU.add)

            # ---- P-D: epilogue over own cols (+pad to yw) ----
            for blk in range((g.yw + bw - 1) // bw):
                o0 = blk * bw
                w = min(bw, g.yw - o0)
                h2t = ldp.tile([64, bw], bf, tag="h2t")
                nc.sync.dma_start(h2t[:, 0:w],
                                  h2pre.ap()[:, g.o0 + o0:g.o0 + o0 + w])
                rhsid = ldp.tile([16, bw], u32, tag="rhsid")
                nc.sync.dma_start(
                    rhsid[:, 0:w],
                    fpu.ap()[:, 3 * g.m + o0:3 * g.m + o0 + w])
                rb = rhsid[:].bitcast(bf).rearrange("p (n two) -> p n two",
                                                    two=2)
                t1 = stgp.tile([64, bw], bf, tag="t1")
                nc.scalar.activation(t1[:, 0:w], h2t[:, 0:w], AF.Relu,
                                     bias=b2[:], scale=a2[:])
                t2 = stgp.tile([64, bw], bf, tag="t2")
                for tl in range(w // til):
                    s0 = tl * til
                    psid = psp.tile([64, til], f32, space="PSUM", tag="ps")
                    nc.tensor.matmul(psid[:], wrsb[0:16, 0:64],
                                     rb[0:16, s0:s0 + til, 0],
                                     start=True, stop=False)
                    nc.tensor.matmul(psid[:], wrsb[0:16, 64:128],
                                     rb[0:16, s0:s0 + til, 1],
                                     start=False, stop=True)
                    nc.vector.scalar_tensor_tensor(
                        out=t2[:, s0:s0 + til], in0=psid[:], scalar=brsb[:],
                        in1=t1[:, s0:s0 + til], op0=ALU.add, op1=ALU.add)
                nc.sync.dma_start(y.ap()[:, o0:o0 + w], t2[:, 0:w])

    nc.compile()
    return nc


# ---------------------------------------------------------------------------
# Host-side preparation
# ---------------------------------------------------------------------------

def _pack_pairs(x):
    """[n, C] f32 -> [C//2, n] uint32 of bf16 (even|odd<<16) pairs."""
    xb = x.astype(BF16)
    lo = xb[:, 0::2].view(np.uint16).astype(np.uint32)
    hi = xb[:, 1::2].view(np.uint16).astype(np.uint32)
    return np.ascontiguousarray((lo | (hi << 16)).T)


def _wrap_idx(iv, g: Geo2):
    """Wrap window-chunk idx values [chunk, 27] -> [128, ch_cols] i16."""
    til = g.til
    ncol = til // 16
    bw = g.bt * til
    cols = []
    for b in range(g.ct // g.bt):
        B = iv[b * bw:(b + 1) * bw]                       # [bw, 27]
        A = B.reshape(bw // 16, 16, g.k).transpose(2, 1, 0)  # [27,16,bw/16]
        Bp = np.zeros((32, 16, bw // 16), np.int16)
        Bp[:g.k] = A
        for gi in range(4):
            cols.append(Bp[gi * 8:(gi + 1) * 8].reshape(128, bw // 16))
    M = np.concatenate(cols, 1)
    # pack odd block's 3-tap tail into even block's tail cols (rows 48:96)
    # so one 96-row gather serves both blocks of a pair
    nc16 = bw // 16
    for b in range(0, g.ct // g.bt - 1, 2):
        a3 = (b * 4 + 3) * nc16
        b3 = ((b + 1) * 4 + 3) * nc16
        M[64:112, a3:a3 + nc16] = M[0:48, b3:b3 + nc16]
    return M


def prepare_inputs(g: Geo2, feats, camera_cond, W1, g1, be1, W2, g2, be2,
                   Wc, bc, Wr, br, nbr, batch_idx):
    n = g.n

    fpg = _pack_pairs(feats)                      # [16, n]

    # --- weight blobs (shared across cores) ---
    W1b = np.asarray(W1, F32)
    wpk1 = np.zeros((128, 512), BF16)
    for gi in range(4):
        for tt in range(G_TAPS[gi]):
            t = gi * 8 + tt
            for par in range(2):
                cb = (gi * 2 + par) * 64
                wpk1[tt * 16:(tt + 1) * 16, cb:cb + 64] = (
                    W1b[t, par::2, :].astype(BF16))
    W2b = np.asarray(W2, F32)
    wpk2 = np.zeros((128, 1024), BF16)
    for gi in range(4):
        for tt in range(G_TAPS[gi]):
            t = gi * 8 + tt
            for q in range(4):
                cb = (gi * 4 + q) * 64
                # partition 16*tt + r holds channel 4r+q of tap t
                wpk2[tt * 16:(tt + 1) * 16, cb:cb + 64] = (
                    W2b[t, q::4, :].astype(BF16))
    # duplicate tail-group weights at rows 48:96 for pair-merged gathers
    wpk1[64:112, 384:512] = wpk1[0:48, 384:512]
    wpk2[64:112, 768:1024] = wpk2[0:48, 768:1024]
    Wrb = np.asarray(Wr, F32)
    wbf = np.zeros((128, 1664), BF16)
    wbf[:, 0:512] = wpk1
    wbf[:, 512:1536] = wpk2
    wbf[0:16, 1536:1600] = Wrb[0::2, :].astype(BF16)
    wbf[0:16, 1600:1664] = Wrb[1::2, :].astype(BF16)

    Wcn = np.asarray(Wc, F32)
    bcn = np.asarray(bc, F32)
    camt = np.concatenate([np.asarray(camera_cond, F32).T,
                           np.ones((1, 4), F32)], 0)
    wf32_base = np.zeros((g.cam + 1, 141), F32)
    wf32_base[:, 0:64] = np.concatenate([Wcn[:, 0:64], bcn[None, 0:64]], 0)
    wf32_base[:, 64:128] = np.concatenate([Wcn[:, 64:128], bcn[None, 64:128]],
                                          0)
    wf32_base[:, 128:132] = camt
    wf32_base[0:64, 136] = np.asarray(g1, F32)
    wf32_base[0:64, 137] = np.asarray(be1, F32)
    wf32_base[0:64, 138] = np.asarray(g2, F32)
    wf32_base[0:64, 139] = np.asarray(be2, F32)
    wf32_base[0:64, 140] = np.asarray(br, F32)

    nbr64 = nbr.astype(np.int64)
    valid_all = nbr64 >= 0

    in_maps = []
    for c in range(g.n_cores):
        own0 = c * g.own
        win0 = own0 - g.o0          # window col 0 <-> global win0
        f0 = win0 - g.m             # fpu col 0 <-> global f0

        fpc = np.zeros((16, g.fwinp), np.uint32)
        lo = max(f0, 0)
        hi = min(f0 + g.fwinp, n)
        if hi > lo:
            fpc[:, lo - f0:hi - f0] = fpg[:, lo:hi]

        Gw = win0 + np.arange(g.win)
        inb = (Gw >= 0) & (Gw < n)
        Gc = np.clip(Gw, 0, n - 1)
        src = np.where((inb[:, None]) & valid_all[Gc], nbr64[Gc], -1)
        blocks = []
        for j in range(g.nch):
            rows = slice(j * g.chunk, (j + 1) * g.chunk)
            sr = src[rows]
            loc = sr - (f0 + j * g.chunk)
            iv = np.where(sr >= 0, loc, g.src - 1)
            assert iv.min() >= 0 and iv.max() <= g.src - 1, (
                f"neighbor offset out of range on core {c}")
            blocks.append(_wrap_idx(iv.astype(np.int16), g))
        idxt = np.concatenate(blocks, 1)
        assert idxt.shape[1] == g.idx_cols

        b = int(batch_idx[own0])
        wf32 = wf32_base.copy()
        wf32[0:64, 132 + b] = 1.0

        in_maps.append({"fpu": fpc, "idx": idxt, "wbf": wbf, "wf32": wf32})
    return in_maps


# ---------------------------------------------------------------------------
# Entry point
# ---------------------------------------------------------------------------

_NC_CACHE = {}


def _get_module(g: Geo2):
    if g not in _NC_CACHE:
        _NC_CACHE[g] = build_module(g)
    return _NC_CACHE[g]


def kernel(**inputs) -> np.ndarray:
    g = GEO2
    nc = _get_module(g)
    args = {k: np.asarray(v) for k, v in inputs.items()}
    in_maps = prepare_inputs(
        g, args["feats"], args["camera_cond"], args["W1"], args["g1"],
        args["be1"], args["W2"], args["g2"], args["be2"], args["Wc"],
        args["bc"], args["Wr"], args["br"], args["nbr"], args["batch_idx"])
    res = run_bass_kernel_spmd(nc, in_maps, core_ids=list(range(g.n_cores)))
    out = np.concatenate(
        [res.results[c]["y"][:, :g.own].T.astype(np.float32)
         for c in range(g.n_cores)], 0)
    return out

